# revision 1
# baseline (speedup 1.0000x reference)
"""Trainium2 Bass kernel v2: batched affine bilinear sampling via dma_gather.

Full inputs: images [32, 512, 512, 3] f32, theta [32, 2, 3] f32.
Data parallel over batch: 8 NeuronCores x 4 images; one SPMD launch per image
(4 output blocks of 128 rows per launch).

Device algorithm per launch:
  1. Stage an fp16 "quad-slot" image in DRAM: slot(y*512+x) = 32B =
     [img[y,x], img[y,x+1], img[y+1,x], img[y+1,x+1]] as 12 fp16 + pad.
     8 slots = one 256B gather element; element index = (y*512+x)>>3 <= 32767
     fits int16.
  2. Per block: compute exact sample coords / lerp weights (baseline DVE
     recipe), plus element index idx = y0*64 + (x0>>3) and slot-in-element
     o = x0 & 7.
  3. Shuffle idx into dma_gather's wrapped index layout
     (table[q, j*8+u] = idx[i=16u+q, j]) via int16 DMA-transpose + a
     strided DRAM round trip, broadcast to all 8 gpsimd index stripes.
  4. 64 dma_gathers per block (1024 idxs each - ucode cap), prepare_only +
     trigger, 8 rotating completion sems; each element lands 8 candidate
     slots per pixel on the pixel's output-row partition.
  5. Mux the right slot with a one-hot multiply + strided tensor_reduce,
     then bilinear-blend with f32 weights and store the block.
"""

import sys
from contextlib import ExitStack

for _p in ("/opt/trn_rl_repo",):
    if _p not in sys.path:
        sys.path.append(_p)

import numpy as np

import concourse.bacc as bacc
import concourse.bass as bass
import concourse.tile as tile
from concourse import library_config, mybir
from concourse.bass_utils import run_bass_kernel_spmd

F32 = mybir.dt.float32
F16 = mybir.dt.float16
I16 = mybir.dt.int16
OP = mybir.AluOpType
ACTF = mybir.ActivationFunctionType
AX = mybir.AxisListType

H = W = 512
P = 128
NBLK = H // P
MAGIC = float(2 ** 23)
N_CORES = 8
BPL = 4            # blocks per launch
NI = 1024          # idxs per dma_gather (ucode cap)
GPB = W // 8       # gathers per block (8 cols each) = 64
NGRP = 8           # mux groups per block (8 gathers / 64 cols each)
NE16 = H * W // 8  # 32768 gather elements


def _floor_exact(nc, pool, v, name):
    """Exact floor of f32 tensor v (|v| < 2^22) -> new tile, via magic round
    + compare fixup. Returns r = floor(v)."""
    r = pool.tile([P, W], F32, name=f"fl_{name}")
    nc.scalar.activation(out=r, in_=v, func=ACTF.Copy, bias=MAGIC)
    nc.scalar.activation(out=r, in_=r, func=ACTF.Copy, bias=-MAGIC)
    g = pool.tile([P, W], F32, name="flg")
    nc.vector.tensor_tensor(out=g, in0=r, in1=v, op=OP.is_gt)
    nc.vector.tensor_sub(r, r, g)
    return r


def _body(ctx: ExitStack, tc: "tile.TileContext", imgs: bass.AP,
          theta: bass.AP, bb: bass.AP, gxr: bass.AP, pr: bass.AP,
          prs: bass.AP, out: bass.AP):
    nc = tc.nc

    imgQ = nc.dram_tensor("imgQ16", [NE16, 128], F16, kind="Internal").ap()

    stg_sem = nc.alloc_semaphore(name="stg_sem")
    tld_sem = nc.alloc_semaphore(name="tld_sem")
    gsems = [nc.alloc_semaphore(name=f"gs{i}") for i in range(16)]

    nc.gpsimd.load_library(library_config.mlp)
    nireg = nc.gpsimd.to_reg(NI)

    const_pool = ctx.enter_context(tc.tile_pool(name="const", bufs=1))
    stage_pool = ctx.enter_context(tc.tile_pool(name="stage", bufs=2))
    pairs_pool = ctx.enter_context(tc.tile_pool(name="pairs", bufs=2))
    coord_pool = ctx.enter_context(tc.tile_pool(name="coord", bufs=1))
    late_pool = ctx.enter_context(tc.tile_pool(name="late", bufs=2))
    tiny_pool = ctx.enter_context(tc.tile_pool(name="tiny", bufs=2))
    tt_pool = ctx.enter_context(tc.tile_pool(name="tt", bufs=2))
    idx_pool = ctx.enter_context(tc.tile_pool(name="idxp", bufs=2))
    pay_pool = ctx.enter_context(tc.tile_pool(name="pay", bufs=2))
    prod_pool = ctx.enter_context(tc.tile_pool(name="prod", bufs=1))
    quad_pool = ctx.enter_context(tc.tile_pool(name="quadm", bufs=1))
    outb_pool = ctx.enter_context(tc.tile_pool(name="outb", bufs=2))
    tabd_pool = ctx.enter_context(tc.tile_pool(name="tabd", bufs=2,
                                               space="DRAM"))

    # --- constants ---
    th = const_pool.tile([P, 6], F32)
    nc.sync.dma_start(out=th, in_=theta.unsqueeze(0).to_broadcast([P, 6]))
    bbs = const_pool.tile([P, BPL], F32)
    nc.sync.dma_start(out=bbs, in_=bb.unsqueeze(0).to_broadcast([P, BPL]))
    gx = const_pool.tile([P, W], F32)   # -1 + j*2/511 ramp on every partition
    nc.sync.dma_start(out=gx, in_=gxr.unsqueeze(0).to_broadcast([P, W]))
    pcol = const_pool.tile([P, 1], F32)  # partition index 0..127
    nc.sync.dma_start(out=pcol, in_=pr.unsqueeze(1))
    pcols = const_pool.tile([P, 1], F32)  # sigma(p) = (p%8)*16 + p//8
    nc.sync.dma_start(out=pcols, in_=prs.unsqueeze(1))

    a_ = th[:, 0:1]; b_ = th[:, 1:2]; c_ = th[:, 2:3]
    d_ = th[:, 3:4]; e_ = th[:, 4:5]; f_ = th[:, 5:6]

    # --- staging: build fp16 quad-slot image in DRAM ---
    imgs_flat = imgs.rearrange("k h w c -> k (h w c)")
    for blk in range(NBLK):
        r0 = blk * P
        loadAB = stage_pool.tile([P, 2, (W + 1) * 3], F32)
        src = bass.AP(
            tensor=imgs_flat.tensor,
            offset=imgs_flat.offset + r0 * W * 3,
            ap=[[W * 3, P], [W * 3, 2], [1, (W + 1) * 3]],
        )
        nc.scalar.dma_start(out=loadAB, in_=src)
        pairs = pairs_pool.tile([P, W, 16], F16)
        flat0 = loadAB[:, 0, :]
        flat1 = loadAB[:, 1, :]
        win0 = bass.AP(tensor=flat0.tensor, offset=flat0.offset,
                       ap=[flat0.ap[0], [3, W], [1, 6]])
        win1 = bass.AP(tensor=flat1.tensor, offset=flat1.offset,
                       ap=[flat1.ap[0], [3, W], [1, 6]])
        win0p = bass.AP(tensor=flat0.tensor, offset=flat0.offset,
                        ap=[flat0.ap[0], [3, W], [1, 4]])
        cv = nc.vector.tensor_copy(out=pairs[:, :, 0:6], in_=win0)
        if blk >= 1:
            cv._wait_ge(stg_sem, 16 * blk)
        cva = nc.scalar.activation(out=pairs[:, :, 6:12], in_=win1,
                                   func=ACTF.Copy)
        cvp = nc.vector.tensor_copy(out=pairs[:, :, 12:16], in_=win0p)
        if blk >= 1:
            cva._wait_ge(stg_sem, 16 * blk)
            cvp._wait_ge(stg_sem, 16 * blk)
        st = nc.gpsimd.dma_start(
            out=bass.AP(tensor=imgQ.tensor, offset=imgQ.offset + r0 * W * 16,
                        ap=[[W * 16, P], [1, W * 16]]),
            in_=pairs.rearrange("p w c -> p (w c)"))
        st.then_inc(stg_sem, 16)

    # --- per-output-block pipeline ---
    A256 = tiny_pool.tile([P, 1], F32, name="A256")
    nc.vector.tensor_scalar_mul(A256, a_, 256.0)
    D256 = tiny_pool.tile([P, 1], F32, name="D256")
    nc.vector.tensor_scalar_mul(D256, d_, 256.0)
    c1x = tiny_pool.tile([P, 1], F32, name="c1x")
    nc.vector.tensor_scalar(out=c1x, in0=c_, scalar1=1.0, scalar2=256.0,
                            op0=OP.add, op1=OP.mult)
    c1y = tiny_pool.tile([P, 1], F32, name="c1y")
    nc.vector.tensor_scalar(out=c1y, in0=f_, scalar1=1.0, scalar2=256.0,
                            op0=OP.add, op1=OP.mult)
    xa = tiny_pool.tile([P, W], F32, name="xa")
    nc.vector.tensor_scalar(out=xa, in0=gx, scalar1=A256, scalar2=None,
                            op0=OP.mult)
    ya = tiny_pool.tile([P, W], F32, name="ya")
    nc.vector.tensor_scalar(out=ya, in0=gx, scalar1=D256, scalar2=None,
                            op0=OP.mult)

    imgQ_ap = bass.AP(tensor=imgQ.tensor, offset=imgQ.offset,
                      ap=[[128, NE16], [1, 128]])

    for q in range(BPL):
        gyb = tiny_pool.tile([P, 1], F32, name="gyb")
        nc.vector.tensor_scalar(out=gyb, in0=pcol, scalar1=512.0 / 511.0,
                                scalar2=bbs[:, q:q + 1], op0=OP.mult,
                                op1=OP.add)
        sx = tiny_pool.tile([P, 1], F32, name="sx")
        nc.vector.tensor_scalar(out=sx, in0=gyb, scalar1=b_, scalar2=c1x,
                                op0=OP.mult, op1=OP.add)
        sy = tiny_pool.tile([P, 1], F32, name="sy")
        nc.vector.tensor_scalar(out=sy, in0=gyb, scalar1=e_, scalar2=c1y,
                                op0=OP.mult, op1=OP.add)

        def coord_side(arow, scol, tag):
            v = late_pool.tile([P, W], F32, name=f"v{tag}")
            nc.vector.tensor_scalar(out=v, in0=arow, scalar1=scol,
                                    scalar2=None, op0=OP.add)
            r = _floor_exact(nc, coord_pool, v, tag)
            nc.vector.tensor_scalar(out=r, in0=r, scalar1=0.0, scalar2=511.0,
                                    op0=OP.max, op1=OP.min)
            p1 = late_pool.tile([P, W], F32, name=f"p1{tag}")
            nc.vector.tensor_scalar(out=p1, in0=r, scalar1=1.0, scalar2=511.0,
                                    op0=OP.add, op1=OP.min)
            nc.vector.tensor_scalar(out=v, in0=v, scalar1=0.0, scalar2=511.0,
                                    op0=OP.max, op1=OP.min)
            nc.vector.tensor_sub(p1, p1, v)   # u0 = x1 - xc
            nc.vector.tensor_sub(v, v, r)     # u1 = xc - x0
            return p1, v, r

        u0, u1, x0f = coord_side(xa, sx, "x")
        v0, v1, y0f = coord_side(ya, sy, "y")

        # o = x0f & 7 (natural row order, for the mux one-hot)
        t8 = coord_pool.tile([P, W], F32, name="t8")
        nc.vector.tensor_scalar(out=t8, in0=x0f, scalar1=0.125,
                                scalar2=MAGIC, op0=OP.mult, op1=OP.add)
        nc.scalar.activation(out=t8, in_=t8, func=ACTF.Copy, bias=-MAGIC)
        fx = coord_pool.tile([P, W], F32, name="fx")
        nc.vector.scalar_tensor_tensor(out=fx, in0=t8, scalar=8.0,
                                       in1=x0f, op0=OP.mult, op1=OP.is_gt)
        nc.vector.tensor_sub(t8, t8, fx)      # t8 = xq
        o = coord_pool.tile([P, W], F32, name="o")
        nc.vector.scalar_tensor_tensor(out=o, in0=t8, scalar=-8.0,
                                       in1=x0f, op0=OP.mult, op1=OP.add)

        # sigma-ordered idx pass: partition p computes row sigma(p), so the
        # transposed idx tile is scatter-contiguous while the gather payload
        # still lands row-natural.
        gybs = tiny_pool.tile([P, 1], F32, name="gybs")
        nc.vector.tensor_scalar(out=gybs, in0=pcols, scalar1=512.0 / 511.0,
                                scalar2=bbs[:, q:q + 1], op0=OP.mult,
                                op1=OP.add)
        sxs = tiny_pool.tile([P, 1], F32, name="sxs")
        nc.vector.tensor_scalar(out=sxs, in0=gybs, scalar1=b_, scalar2=c1x,
                                op0=OP.mult, op1=OP.add)
        sys_ = tiny_pool.tile([P, 1], F32, name="sys")
        nc.vector.tensor_scalar(out=sys_, in0=gybs, scalar1=e_, scalar2=c1y,
                                op0=OP.mult, op1=OP.add)

        def floor_clamp_s(arow, scol, tag):
            v = coord_pool.tile([P, W], F32, name=f"vs{tag}")
            nc.vector.tensor_scalar(out=v, in0=arow, scalar1=scol,
                                    scalar2=None, op0=OP.add)
            r = _floor_exact(nc, coord_pool, v, f"s{tag}")
            nc.vector.tensor_scalar(out=r, in0=r, scalar1=0.0, scalar2=511.0,
                                    op0=OP.max, op1=OP.min)
            return r

        x0s = floor_clamp_s(xa, sxs, "x")
        y0s = floor_clamp_s(ya, sys_, "y")
        t8s = coord_pool.tile([P, W], F32, name="t8s")
        nc.vector.tensor_scalar(out=t8s, in0=x0s, scalar1=0.125,
                                scalar2=MAGIC, op0=OP.mult, op1=OP.add)
        nc.scalar.activation(out=t8s, in_=t8s, func=ACTF.Copy, bias=-MAGIC)
        fxs = coord_pool.tile([P, W], F32, name="fxs")
        nc.vector.scalar_tensor_tensor(out=fxs, in0=t8s, scalar=8.0,
                                       in1=x0s, op0=OP.mult, op1=OP.is_gt)
        nc.vector.tensor_sub(t8s, t8s, fxs)   # xq (sigma order)
        idxf = coord_pool.tile([P, W], F32, name="idxf")
        nc.vector.scalar_tensor_tensor(out=idxf, in0=y0s, scalar=64.0,
                                       in1=t8s, op0=OP.mult, op1=OP.add)
        idx16 = coord_pool.tile([P, W], I16, name="idx16")
        nc.vector.tensor_copy(out=idx16, in_=idxf)

        # one-hot of o: oh[p, j, s] = (o == s), fp16
        oh = late_pool.tile([P, W, 8], F16, name="oh")
        for s in range(8):
            nc.vector.tensor_scalar(out=oh[:, :, s], in0=o,
                                    scalar1=float(s), scalar2=None,
                                    op0=OP.is_equal)

        # --- idx shuffle into wrapped layout via transpose + DRAM trip ---
        tD = tabd_pool.tile([16, W * 8], I16, name="tabD")
        tT = tt_pool.tile([P, W], I16, name="tT")
        for c in range(W // P):
            nc.sync.dma_start_transpose(
                out=tT[:, c * P:(c + 1) * P],
                in_=idx16[:, c * P:(c + 1) * P])
        for c in range(W // P):
            nc.scalar.dma_start(
                out=bass.AP(tensor=tD.tensor, offset=tD.offset + c * P * 8,
                            ap=[[8, P], [W * 8, 16], [1, 8]]),
                in_=tT[:, c * P:(c + 1) * P])
        idx_sb = idx_pool.tile([P, W * 8], I16, name="idxsb")
        ld = nc.gpsimd.dma_start(
            out=idx_sb,
            in_=bass.AP(tensor=tD.tensor, offset=tD.offset,
                        ap=[[0, 8], [W * 8, 16], [1, W * 8]]))
        ld.then_inc(tld_sem, 16)

        # --- gathers + mux + blend per 64-col group ---
        outblk = outb_pool.tile([P, W, 3], F32, name="outblk")
        for grp in range(NGRP):
            gg = q * NGRP + grp  # global group id
            pay = pay_pool.tile([P, 64, 128], F16, name="pay")
            for l in range(8):
                g = grp * 8 + l
                prep = nc.gpsimd.dma_gather(
                    out_ap=pay[:, l * 8:(l + 1) * 8, :],
                    in_ap=imgQ_ap,
                    idxs_ap=idx_sb[:, g * 64:(g + 1) * 64],
                    num_idxs=NI,
                    num_idxs_reg=nireg,
                    elem_size=128,
                    prepare_only=True,
                    sem=gsems[(gg * 8 + l) % 16],
                )
                if grp == 0 and l == 0:
                    # Pool program order makes this transitively guard all
                    # later preps of the block
                    prep._wait_ge(tld_sem, 16 * (q + 1))
                trig = nc.gpsimd.trigger_dma(count=None)
                ggl = gg * 8 + l
                lane = ggl % 16
                if ggl == 0:
                    trig._wait_ge(stg_sem, 64)
                elif ggl >= 16:
                    trig._wait_ge(gsems[lane], 16 * (ggl // 16))
            sl = slice(grp * 64, (grp + 1) * 64)
            prod = prod_pool.tile([P, 64, 12, 8], F16, name="prod")
            for l in range(8):
                ohb = bass.AP(tensor=oh.tensor,
                              offset=oh.offset + (grp * 64 + l * 8) * 8,
                              ap=[oh.ap[0], [8, 8], [0, 12], [1, 8]])
                payb = bass.AP(tensor=pay.tensor,
                               offset=pay.offset + l * 8 * 128,
                               ap=[pay.ap[0], [128, 8], [1, 12], [16, 8]])
                prodb = bass.AP(tensor=prod.tensor,
                                offset=prod.offset + l * 8 * 96,
                                ap=[prod.ap[0], [96, 8], [8, 12], [1, 8]])
                pm = nc.vector.tensor_mul(prodb, ohb, payb)
                ggl = gg * 8 + l
                pm._wait_ge(gsems[ggl % 16], 16 * (ggl // 16 + 1))
            quadm = quad_pool.tile([P, 64, 12], F32, name="quadm")
            nc.vector.tensor_reduce(out=quadm, in_=prod, axis=AX.X,
                                    op=OP.add)
            # blend: out = v0*(u0*P00 + u1*P01) + v1*(u0*P10 + u1*P11)
            def wb(wt):
                return bass.AP(tensor=wt.tensor,
                               offset=wt.offset + grp * 64,
                               ap=[wt.ap[0], [1, 64], [0, 3]])
            acc0 = quad_pool.tile([P, 64, 3], F32, name="acc0")
            tmp = quad_pool.tile([P, 64, 3], F32, name="tmpb")
            nc.vector.tensor_mul(acc0, quadm[:, :, 0:3], wb(u0))
            nc.vector.tensor_mul(tmp, quadm[:, :, 3:6], wb(u1))
            nc.vector.tensor_add(acc0, acc0, tmp)
            acc1 = quad_pool.tile([P, 64, 3], F32, name="acc1")
            nc.vector.tensor_mul(acc1, quadm[:, :, 6:9], wb(u0))
            nc.vector.tensor_mul(tmp, quadm[:, :, 9:12], wb(u1))
            nc.vector.tensor_add(acc1, acc1, tmp)
            nc.vector.tensor_mul(acc0, acc0, wb(v0))
            nc.vector.tensor_mul(acc1, acc1, wb(v1))
            nc.vector.tensor_add(outblk[:, sl, :], acc0, acc1)
        nc.sync.dma_start(out=out[q], in_=outblk)


def build_kernel2(num_devices: int = N_CORES):
    nc = bacc.Bacc("TRN2", target_bir_lowering=False, debug=False,
                   num_devices=num_devices)
    imgs = nc.dram_tensor("imgs", [1, H + 2, W, 3], F32, kind="ExternalInput")
    theta = nc.dram_tensor("theta", [6], F32, kind="ExternalInput")
    bb = nc.dram_tensor("bb", [BPL], F32, kind="ExternalInput")
    gxr = nc.dram_tensor("gxr", [W], F32, kind="ExternalInput")
    pr = nc.dram_tensor("pr", [P], F32, kind="ExternalInput")
    prs = nc.dram_tensor("prs", [P], F32, kind="ExternalInput")
    out = nc.dram_tensor("out", [BPL, P, W, 3], F32, kind="ExternalOutput")
    with tile.TileContext(nc) as tc:
        with ExitStack() as ctx:
            _body(ctx, tc, imgs.ap(), theta.ap(), bb.ap(), gxr.ap(), pr.ap(),
                  prs.ap(), out.ap())
    nc.compile()
    return nc


_NC_CACHE = {}


def run_kernel_spmd(images: np.ndarray, theta: np.ndarray, trace: bool = False):
    B = images.shape[0]
    per = B // N_CORES
    if "k2" not in _NC_CACHE:
        _NC_CACHE["k2"] = build_kernel2(N_CORES)
    nc = _NC_CACHE["k2"]

    out = np.zeros((B, H, W, 3), np.float32)
    slabs = []
    for c in range(N_CORES):
        s = np.zeros((per, H + 2, W, 3), np.float32)
        s[:, :H] = images[c * per:(c + 1) * per]
        slabs.append(s)

    gxr = (np.arange(W, dtype=np.float32) * (2.0 / 511.0) - 1.0).astype(
        np.float32)
    prv = np.arange(P, dtype=np.float32)
    pi = np.arange(P)
    prsv = ((pi % 8) * 16 + pi // 8).astype(np.float32)
    bbv = np.array([128.0 * q * (512.0 / 511.0) - 256.0 for q in range(BPL)],
                   np.float32)

    last_res = None
    for k in range(per):
        in_maps = []
        for c in range(N_CORES):
            in_maps.append({
                "imgs": slabs[c][k:k + 1],
                "theta": np.ascontiguousarray(
                    theta[c * per + k].reshape(-1)).astype(np.float32),
                "bb": bbv,
                "gxr": gxr,
                "pr": prv,
                "prs": prsv,
            })
        res = run_bass_kernel_spmd(nc, in_maps, core_ids=list(range(N_CORES)),
                                   trace=trace)
        last_res = res
        for c in range(N_CORES):
            out[c * per + k] = res.results[c]["out"].reshape(H, W, 3)
    return out, last_res


def kernel(images: np.ndarray, theta: np.ndarray) -> np.ndarray:
    images = np.ascontiguousarray(np.asarray(images), dtype=np.float32)
    theta = np.asarray(theta).astype(np.float32)
    out, _ = run_kernel_spmd(images, theta, trace=False)
    return out



# revision 37
# speedup vs baseline: 1.6622x; 1.6622x over previous
"""Trainium2 Bass kernel v3: batched affine bilinear sampling via dma_gather.

Full inputs: images [32, 512, 512, 3] f32, theta [32, 2, 3] f32.
Data parallel over batch: 8 NeuronCores x 4 images; one SPMD launch per image
(4 output blocks of 128 rows per launch).

v3 vs v2: the gather element is VALUE-MAJOR so the slot dim (s = x&7) is
packed innermost: element for (y, octet k) holds 128 fp16 at position
v*8 + s with v = c*4 + tap, tap = dy*2+dx (c=3 lane is junk padding).
This lets the 8-way slot mux run as ONE fused fp16 tensor_tensor multiply
per 64-column group (DVE 2x mode) followed by a packed pairwise add tree,
and the bilinear blend as a second fused multiply + small add tree.
DMA issue for staging/idx/output moved off the Pool engine (gather preps
own it).

Device algorithm per launch:
  1. Stage the fp16 value-major element image in DRAM: 32768 elements of
     256B; element index = y*64 + (x>>3) <= 32767 fits int16.
  2. Per 128-row output block: compute sample coords / lerp weights, the
     element index idx = y0*64 + (x0>>3), slot o = x0 & 7, onehot(o) and
     the 4 bilinear tap weights W4 = [u0v0, u1v0, u0v1, u1v1] (fp16).
  3. Shuffle idx into dma_gather's wrapped index layout via int16
     DMA-transpose + a strided DRAM round trip, broadcast to all 8 gpsimd
     index stripes.
  4. 64 dma_gathers per block (1024 idxs each - ucode cap), prepare_only +
     trigger, 4 rotating per-group completion sems (2 groups in flight).
  5. Per group: prod = pay * onehot (fp16 2x), add tree 8->4->2->1 to mux
     the right slot, then blend via quadm * W4 + add tree 4->2->1 -> f32.
"""

import sys
from contextlib import ExitStack

for _p in ("/opt/trn_rl_repo",):
    if _p not in sys.path:
        sys.path.append(_p)

import numpy as np

import concourse.bacc as bacc
import concourse.bass as bass
import concourse.tile as tile
from concourse import library_config, mybir
from concourse.bass_utils import run_bass_kernel_spmd

F32 = mybir.dt.float32
F16 = mybir.dt.float16
I16 = mybir.dt.int16
OP = mybir.AluOpType
ACTF = mybir.ActivationFunctionType
AX = mybir.AxisListType

H = W = 512
P = 128
NBLK = H // P
MAGIC = float(2 ** 23)
N_CORES = 8
BPL = 4            # blocks per launch
NI = 1024          # idxs per dma_gather (ucode cap)
NGRP = 8           # groups per block (8 gathers / 64 out cols each)
NE16 = H * W // 8  # 32768 gather elements
NGS = 4            # rotating group-completion sems
ROWB = (W + 2) * 3  # staged row bytes-in-f32-elems (x and channel slack)


def _floor_exact(nc, pool, v, name):
    """floor of f32 tensor v via round-half-down magic (bias M - 0.5), no
    DVE fixup. At exact-integer ties v == k with k odd this returns k - 1
    instead of k, but the bilinear weights then put all mass on the x1 tap
    which IS k, so the sampled value is identical (verified against the
    fixed harness inputs)."""
    r = pool.tile([P, W], F32, name=f"fl_{name}")
    nc.scalar.activation(out=r, in_=v, func=ACTF.Copy, bias=MAGIC - 0.5)
    nc.scalar.activation(out=r, in_=r, func=ACTF.Copy, bias=-MAGIC)
    return r


def _body(ctx: ExitStack, tc: "tile.TileContext", imgs: bass.AP,
          theta: bass.AP, bb: bass.AP, gxr: bass.AP, pr: bass.AP,
          prs: bass.AP, out: bass.AP):
    nc = tc.nc

    imgQ = nc.dram_tensor("imgQ16", [NE16, 128], F16, kind="Internal").ap()

    stg_sem = nc.alloc_semaphore(name="stg_sem")
    gsems = [nc.alloc_semaphore(name=f"gs{i}") for i in range(NGS)]

    nc.gpsimd.load_library(library_config.mlp)
    nireg = nc.gpsimd.to_reg(NI)

    const_pool = ctx.enter_context(tc.tile_pool(name="const", bufs=1))
    stage_pool = ctx.enter_context(tc.tile_pool(name="stage", bufs=1))
    pairs_pool = ctx.enter_context(tc.tile_pool(name="pairs", bufs=2))
    coord_pool = ctx.enter_context(tc.tile_pool(name="coord", bufs=1))
    late_pool = ctx.enter_context(tc.tile_pool(name="late", bufs=2))
    tiny_pool = ctx.enter_context(tc.tile_pool(name="tiny", bufs=2))
    tt_pool = ctx.enter_context(tc.tile_pool(name="tt", bufs=2))
    idx_pool = ctx.enter_context(tc.tile_pool(name="idxp", bufs=BPL))
    pay_pool = ctx.enter_context(tc.tile_pool(name="pay", bufs=2))
    prod_pool = ctx.enter_context(tc.tile_pool(name="prod", bufs=1))
    quad_pool = ctx.enter_context(tc.tile_pool(name="quadm", bufs=1))
    outb_pool = ctx.enter_context(tc.tile_pool(name="outb", bufs=2))
    tabd_pool = ctx.enter_context(tc.tile_pool(name="tabd", bufs=BPL,
                                               space="DRAM"))

    # --- constants ---
    th = const_pool.tile([P, 6], F32)
    nc.sync.dma_start(out=th, in_=theta.unsqueeze(0).to_broadcast([P, 6]))
    bbs = const_pool.tile([P, BPL], F32)
    nc.sync.dma_start(out=bbs, in_=bb.unsqueeze(0).to_broadcast([P, BPL]))
    gx = const_pool.tile([P, W], F32)   # -1 + j*2/511 ramp on every partition
    nc.sync.dma_start(out=gx, in_=gxr.unsqueeze(0).to_broadcast([P, W]))
    pcol = const_pool.tile([P, 1], F32)  # partition index 0..127
    nc.sync.dma_start(out=pcol, in_=pr.unsqueeze(1))
    pcols = const_pool.tile([P, 1], F32)  # sigma(p) = (p%8)*16 + p//8
    nc.sync.dma_start(out=pcols, in_=prs.unsqueeze(1))

    a_ = th[:, 0:1]; b_ = th[:, 1:2]; c_ = th[:, 2:3]
    d_ = th[:, 3:4]; e_ = th[:, 4:5]; f_ = th[:, 5:6]

    # --- staging: build fp16 value-major element image in DRAM ---
    # element e = y*64 + k holds fp16[v*8 + s] = img[y+dy, 8k+s+dx, c]
    # with v = c*4 + (dy*2+dx); v/8 lanes for c==3 are junk (weight-0 side
    # reads of the next pixel's channel 0).
    imgs_flat = imgs.rearrange("k h w c -> k (h w c)")
    for blk in range(NBLK):
        r0 = blk * P
        loadAB = stage_pool.tile([P, 2, ROWB], F32)
        src = bass.AP(
            tensor=imgs_flat.tensor,
            offset=imgs_flat.offset + r0 * W * 3,
            ap=[[W * 3, P], [W * 3, 2], [1, ROWB]],
        )
        nc.scalar.dma_start(out=loadAB, in_=src)
        pairs = pairs_pool.tile([P, 64, 128], F16)
        ops = []
        for t in range(4):
            dy, dx = t >> 1, t & 1
            flat = loadAB[:, dy, :]
            # src: [c(4), k(64), s(8)] strides in f32 elems: c:1, k:24, s:3
            sap = bass.AP(tensor=flat.tensor, offset=flat.offset + dx * 3,
                          ap=[flat.ap[0], [1, 4], [24, 64], [3, 8]])
            # dst: pairs[p, k, (c*4+t)*8 + s] -> c:32, k:128, s:1
            dap = bass.AP(tensor=pairs.tensor, offset=pairs.offset + t * 8,
                          ap=[pairs.ap[0], [32, 4], [128, 64], [1, 8]])
            op = nc.scalar.activation(out=dap, in_=sap, func=ACTF.Copy)
            ops.append(op)
        if blk >= 1:
            # WAR vs the gpsimd stores of earlier pairs buffers
            for op in ops:
                op._wait_ge(stg_sem, 16 * blk)
        st = nc.gpsimd.dma_start(
            out=bass.AP(tensor=imgQ.tensor, offset=imgQ.offset + r0 * W * 16,
                        ap=[[W * 16, P], [1, W * 16]]),
            in_=pairs.rearrange("p w c -> p (w c)"))
        st.then_inc(stg_sem, 16)

    # --- per-output-block pipeline ---
    A256 = tiny_pool.tile([P, 1], F32, name="A256")
    nc.vector.tensor_scalar_mul(A256, a_, 256.0)
    D256 = tiny_pool.tile([P, 1], F32, name="D256")
    nc.vector.tensor_scalar_mul(D256, d_, 256.0)
    c1x = tiny_pool.tile([P, 1], F32, name="c1x")
    nc.vector.tensor_scalar(out=c1x, in0=c_, scalar1=1.0, scalar2=256.0,
                            op0=OP.add, op1=OP.mult)
    c1y = tiny_pool.tile([P, 1], F32, name="c1y")
    nc.vector.tensor_scalar(out=c1y, in0=f_, scalar1=1.0, scalar2=256.0,
                            op0=OP.add, op1=OP.mult)
    xa = tiny_pool.tile([P, W], F32, name="xa")
    nc.vector.tensor_scalar(out=xa, in0=gx, scalar1=A256, scalar2=None,
                            op0=OP.mult)
    ya = tiny_pool.tile([P, W], F32, name="ya")
    nc.vector.tensor_scalar(out=ya, in0=gx, scalar1=D256, scalar2=None,
                            op0=OP.mult)

    imgQ_ap = bass.AP(tensor=imgQ.tensor, offset=imgQ.offset,
                      ap=[[128, NE16], [1, 128]])

    for q in range(BPL):
        gyb = tiny_pool.tile([P, 1], F32, name="gyb")
        nc.vector.tensor_scalar(out=gyb, in0=pcol, scalar1=512.0 / 511.0,
                                scalar2=bbs[:, q:q + 1], op0=OP.mult,
                                op1=OP.add)
        sx = tiny_pool.tile([P, 1], F32, name="sx")
        nc.vector.tensor_scalar(out=sx, in0=gyb, scalar1=b_, scalar2=c1x,
                                op0=OP.mult, op1=OP.add)
        sy = tiny_pool.tile([P, 1], F32, name="sy")
        nc.vector.tensor_scalar(out=sy, in0=gyb, scalar1=e_, scalar2=c1y,
                                op0=OP.mult, op1=OP.add)

        def coord_side(arow, scol, tag):
            v = coord_pool.tile([P, W], F32, name=f"v{tag}")
            nc.vector.tensor_scalar(out=v, in0=arow, scalar1=scol,
                                    scalar2=None, op0=OP.add)
            # m = (v < 511): at exactly v == 511 the reference zeroes both
            # weights but half-down floor gives r=510/u1=1; mask u1 to 0.
            m = coord_pool.tile([P, W], F32, name=f"m{tag}")
            nc.vector.tensor_scalar(out=m, in0=v, scalar1=511.0,
                                    scalar2=None, op0=OP.is_lt)
            r = _floor_exact(nc, coord_pool, v, tag)
            nc.vector.tensor_scalar(out=r, in0=r, scalar1=0.0, scalar2=511.0,
                                    op0=OP.max, op1=OP.min)
            p1 = coord_pool.tile([P, W], F32, name=f"p1{tag}")
            nc.vector.tensor_scalar(out=p1, in0=r, scalar1=1.0, scalar2=511.0,
                                    op0=OP.add, op1=OP.min)
            nc.vector.tensor_scalar(out=v, in0=v, scalar1=0.0, scalar2=511.0,
                                    op0=OP.max, op1=OP.min)
            nc.vector.tensor_sub(p1, p1, v)   # u0 = x1 - xc
            nc.vector.tensor_sub(v, v, r)     # u1 = xc - x0
            nc.vector.tensor_mul(v, v, m)     # kill u1 at v >= 511 exactly
            return p1, v, r

        u0, u1, x0f = coord_side(xa, sx, "x")
        v0, v1, y0f = coord_side(ya, sy, "y")

        # o = x0f & 7 (natural row order, for the mux one-hot)
        t8 = coord_pool.tile([P, W], F32, name="t8")
        nc.vector.tensor_scalar(out=t8, in0=x0f, scalar1=0.125,
                                scalar2=MAGIC, op0=OP.mult, op1=OP.add)
        nc.scalar.activation(out=t8, in_=t8, func=ACTF.Copy, bias=-MAGIC)
        fx = coord_pool.tile([P, W], F32, name="fx")
        nc.vector.scalar_tensor_tensor(out=fx, in0=t8, scalar=8.0,
                                       in1=x0f, op0=OP.mult, op1=OP.is_gt)
        nc.vector.tensor_sub(t8, t8, fx)      # t8 = xq
        o = coord_pool.tile([P, W], F32, name="o")
        nc.vector.scalar_tensor_tensor(out=o, in0=t8, scalar=-8.0,
                                       in1=x0f, op0=OP.mult, op1=OP.add)

        # one-hot of o: oh[p, j, s] = (o == s), fp16, s packed innermost
        oh = late_pool.tile([P, W, 8], F16, name="oh")
        for s in range(8):
            nc.vector.tensor_scalar(out=oh[:, :, s], in0=o,
                                    scalar1=float(s), scalar2=None,
                                    op0=OP.is_equal)

        # W4[p, j, t] = bilinear tap weights (fp16): t0=u0v0 t1=u1v0
        # t2=u0v1 t3=u1v1
        w4 = late_pool.tile([P, W, 4], F16, name="w4")
        nc.vector.tensor_mul(w4[:, :, 0], u0, v0)
        nc.vector.tensor_mul(w4[:, :, 1], u1, v0)
        nc.vector.tensor_mul(w4[:, :, 2], u0, v1)
        nc.vector.tensor_mul(w4[:, :, 3], u1, v1)

        # idx in NATURAL row order from already-computed y0f/t8:
        # idx = y0 * 64 + (x0 >> 3)
        idxf = coord_pool.tile([P, W], F32, name="idxf")
        nc.vector.scalar_tensor_tensor(out=idxf, in0=y0f, scalar=64.0,
                                       in1=t8, op0=OP.mult, op1=OP.add)
        idx16 = coord_pool.tile([P, W], I16, name="idx16")
        nc.vector.tensor_copy(out=idx16, in_=idxf)

        # --- idx shuffle into wrapped layout via transpose + DRAM trip ---
        # tT[a, c*128+b] = idx of pixel (row b, col c*128+a). The wrapped
        # table needs tD[q][j*8+u] = idx of pixel (16u+q, j); reading tT's
        # free dim as b = i1 + 16*i2 (strides [1,16],[16,8]) delivers rows
        # in exactly that order, so no sigma-ordered second coord pass.
        tD = tabd_pool.tile([16, W * 8], I16, name="tabD")
        tT = tt_pool.tile([P, W], I16, name="tT")
        for c in range(W // P):
            nc.sync.dma_start_transpose(
                out=tT[:, c * P:(c + 1) * P],
                in_=idx16[:, c * P:(c + 1) * P])
        for c in range(W // P):
            nc.scalar.dma_start(
                out=bass.AP(tensor=tD.tensor, offset=tD.offset + c * P * 8,
                            ap=[[8, P], [1, 8], [W * 8, 16]]),
                in_=tT[:, c * P:(c + 1) * P])
        # idx bufs == BPL so there is no buffer reuse within a launch; the
        # tile framework tracks this SP DMA write -> gather-prep read dep.
        idx_sb = idx_pool.tile([P, W * 8], I16, name="idxsb")
        nc.sync.dma_start(
            out=idx_sb,
            in_=bass.AP(tensor=tD.tensor, offset=tD.offset,
                        ap=[[0, 8], [W * 8, 16], [1, W * 8]]))

        # --- gathers + fused mux + blend per 64-col group ---
        outblk = outb_pool.tile([P, W, 3], F32, name="outblk")
        for grp in range(NGRP):
            gg = q * NGRP + grp  # global group id
            gsem = gsems[gg % NGS]
            pay = pay_pool.tile([P, 64, 128], F16, name="pay")
            for l in range(8):
                g = grp * 8 + l
                prep = nc.gpsimd.dma_gather(
                    out_ap=pay[:, l * 8:(l + 1) * 8, :],
                    in_ap=imgQ_ap,
                    idxs_ap=idx_sb[:, g * 64:(g + 1) * 64],
                    num_idxs=NI,
                    num_idxs_reg=nireg,
                    elem_size=128,
                    prepare_only=True,
                    sem=gsem,
                )
                trig = nc.gpsimd.trigger_dma(count=None)
                if gg == 0 and l == 0:
                    trig._wait_ge(stg_sem, 16 * NBLK)
                elif gg >= 2 and l == 0:
                    pg = gg - 2
                    trig._wait_ge(gsems[pg % NGS], 128 * (pg // NGS + 1))
            sl = slice(grp * 64, (grp + 1) * 64)
            # fused mux: prod[p, j, v, s] = pay * oh (fp16 2x), then add
            # tree over s: 8 -> 4 -> 2 -> 1
            prod = prod_pool.tile([P, 64, 12, 8], F16, name="prod")
            payv = bass.AP(tensor=pay.tensor, offset=pay.offset,
                           ap=[pay.ap[0], [128, 64], [8, 12], [1, 8]])
            ohv = bass.AP(tensor=oh.tensor, offset=oh.offset + grp * 64 * 8,
                          ap=[oh.ap[0], [8, 64], [0, 12], [1, 8]])
            pm = nc.vector.tensor_mul(prod, payv, ohv)
            pm._wait_ge(gsem, 128 * (gg // NGS + 1))
            sum4 = prod_pool.tile([P, 64, 12, 4], F16, name="sum4")
            p0 = bass.AP(tensor=prod.tensor, offset=prod.offset,
                         ap=[prod.ap[0], [96, 64], [8, 12], [1, 4]])
            p1 = bass.AP(tensor=prod.tensor, offset=prod.offset + 4,
                         ap=[prod.ap[0], [96, 64], [8, 12], [1, 4]])
            nc.vector.tensor_add(sum4, p0, p1)
            sum2 = prod_pool.tile([P, 64, 12, 2], F16, name="sum2")
            s40 = bass.AP(tensor=sum4.tensor, offset=sum4.offset,
                          ap=[sum4.ap[0], [48, 64], [4, 12], [1, 2]])
            s41 = bass.AP(tensor=sum4.tensor, offset=sum4.offset + 2,
                          ap=[sum4.ap[0], [48, 64], [4, 12], [1, 2]])
            nc.vector.tensor_add(sum2, s40, s41)
            quadm = quad_pool.tile([P, 64, 12], F16, name="quadm")
            s20 = bass.AP(tensor=sum2.tensor, offset=sum2.offset,
                          ap=[sum2.ap[0], [24, 64], [2, 12]])
            s21 = bass.AP(tensor=sum2.tensor, offset=sum2.offset + 1,
                          ap=[sum2.ap[0], [24, 64], [2, 12]])
            nc.vector.tensor_add(quadm, s20, s21)
            # blend: bmul[p, j, c, t] = quadm * W4, add tree 4 -> 2 -> 1
            bmul = quad_pool.tile([P, 64, 3, 4], F16, name="bmul")
            qv = bass.AP(tensor=quadm.tensor, offset=quadm.offset,
                         ap=[quadm.ap[0], [12, 64], [4, 3], [1, 4]])
            wv = bass.AP(tensor=w4.tensor, offset=w4.offset + grp * 64 * 4,
                         ap=[w4.ap[0], [4, 64], [0, 3], [1, 4]])
            nc.vector.tensor_mul(bmul, qv, wv)
            b2 = quad_pool.tile([P, 64, 3, 2], F16, name="b2")
            b20 = bass.AP(tensor=bmul.tensor, offset=bmul.offset,
                          ap=[bmul.ap[0], [12, 64], [4, 3], [1, 2]])
            b21 = bass.AP(tensor=bmul.tensor, offset=bmul.offset + 2,
                          ap=[bmul.ap[0], [12, 64], [4, 3], [1, 2]])
            nc.vector.tensor_add(b2, b20, b21)
            ob = bass.AP(tensor=outblk.tensor,
                         offset=outblk.offset + grp * 64 * 3,
                         ap=[outblk.ap[0], [3, 64], [1, 3]])
            b2e = bass.AP(tensor=b2.tensor, offset=b2.offset,
                          ap=[b2.ap[0], [6, 64], [2, 3]])
            b2o = bass.AP(tensor=b2.tensor, offset=b2.offset + 1,
                          ap=[b2.ap[0], [6, 64], [2, 3]])
            nc.vector.tensor_add(ob, b2e, b2o)
        nc.sync.dma_start(out=out[q], in_=outblk)


def build_kernel2(num_devices: int = N_CORES):
    nc = bacc.Bacc("TRN2", target_bir_lowering=False, debug=False,
                   num_devices=num_devices)
    imgs = nc.dram_tensor("imgs", [1, H + 2, W, 3], F32, kind="ExternalInput")
    theta = nc.dram_tensor("theta", [6], F32, kind="ExternalInput")
    bb = nc.dram_tensor("bb", [BPL], F32, kind="ExternalInput")
    gxr = nc.dram_tensor("gxr", [W], F32, kind="ExternalInput")
    pr = nc.dram_tensor("pr", [P], F32, kind="ExternalInput")
    prs = nc.dram_tensor("prs", [P], F32, kind="ExternalInput")
    out = nc.dram_tensor("out", [BPL, P, W, 3], F32, kind="ExternalOutput")
    with tile.TileContext(nc) as tc:
        with ExitStack() as ctx:
            _body(ctx, tc, imgs.ap(), theta.ap(), bb.ap(), gxr.ap(), pr.ap(),
                  prs.ap(), out.ap())
    nc.compile()
    return nc


_NC_CACHE = {}


def run_kernel_spmd(images: np.ndarray, theta: np.ndarray, trace: bool = False):
    B = images.shape[0]
    per = B // N_CORES
    if "k3" not in _NC_CACHE:
        _NC_CACHE["k3"] = build_kernel2(N_CORES)
    nc = _NC_CACHE["k3"]

    out = np.zeros((B, H, W, 3), np.float32)
    slabs = []
    for c in range(N_CORES):
        s = np.zeros((per, H + 2, W, 3), np.float32)
        s[:, :H] = images[c * per:(c + 1) * per]
        slabs.append(s)

    gxr = (np.arange(W, dtype=np.float32) * (2.0 / 511.0) - 1.0).astype(
        np.float32)
    prv = np.arange(P, dtype=np.float32)
    pi = np.arange(P)
    prsv = ((pi % 8) * 16 + pi // 8).astype(np.float32)
    bbv = np.array([128.0 * q * (512.0 / 511.0) - 256.0 for q in range(BPL)],
                   np.float32)

    last_res = None
    for k in range(per):
        in_maps = []
        for c in range(N_CORES):
            in_maps.append({
                "imgs": slabs[c][k:k + 1],
                "theta": np.ascontiguousarray(
                    theta[c * per + k].reshape(-1)).astype(np.float32),
                "bb": bbv,
                "gxr": gxr,
                "pr": prv,
                "prs": prsv,
            })
        res = run_bass_kernel_spmd(nc, in_maps, core_ids=list(range(N_CORES)),
                                   trace=trace)
        last_res = res
        for c in range(N_CORES):
            out[c * per + k] = res.results[c]["out"].reshape(H, W, 3)
    return out, last_res


def kernel(images: np.ndarray, theta: np.ndarray) -> np.ndarray:
    images = np.ascontiguousarray(np.asarray(images), dtype=np.float32)
    theta = np.asarray(theta).astype(np.float32)
    out, _ = run_kernel_spmd(images, theta, trace=False)
    return out


# revision 45
# speedup vs baseline: 1.7385x; 1.0459x over previous
"""Trainium2 Bass kernel v3: batched affine bilinear sampling via dma_gather.

Full inputs: images [32, 512, 512, 3] f32, theta [32, 2, 3] f32.
Data parallel over batch: 8 NeuronCores x 4 images; one SPMD launch per image
(4 output blocks of 128 rows per launch).

v3 vs v2: the gather element is VALUE-MAJOR so the slot dim (s = x&7) is
packed innermost: element for (y, octet k) holds 128 fp16 at position
v*8 + s with v = c*4 + tap, tap = dy*2+dx (c=3 lane is junk padding).
This lets the 8-way slot mux run as ONE fused fp16 tensor_tensor multiply
per 64-column group (DVE 2x mode) followed by a packed pairwise add tree,
and the bilinear blend as a second fused multiply + small add tree.
DMA issue for staging/idx/output moved off the Pool engine (gather preps
own it).

Device algorithm per launch:
  1. Stage the fp16 value-major element image in DRAM: 32768 elements of
     256B; element index = y*64 + (x>>3) <= 32767 fits int16.
  2. Per 128-row output block: compute sample coords / lerp weights, the
     element index idx = y0*64 + (x0>>3), slot o = x0 & 7, onehot(o) and
     the 4 bilinear tap weights W4 = [u0v0, u1v0, u0v1, u1v1] (fp16).
  3. Shuffle idx into dma_gather's wrapped index layout via int16
     DMA-transpose + a strided DRAM round trip, broadcast to all 8 gpsimd
     index stripes.
  4. 64 dma_gathers per block (1024 idxs each - ucode cap), prepare_only +
     trigger, 4 rotating per-group completion sems (2 groups in flight).
  5. Per group: prod = pay * onehot (fp16 2x), add tree 8->4->2->1 to mux
     the right slot, then blend via quadm * W4 + add tree 4->2->1 -> f32.
"""

import sys
from contextlib import ExitStack

for _p in ("/opt/trn_rl_repo",):
    if _p not in sys.path:
        sys.path.append(_p)

import numpy as np

import concourse.bacc as bacc
import concourse.bass as bass
import concourse.tile as tile
from concourse import library_config, mybir
from concourse.bass_utils import run_bass_kernel_spmd

F32 = mybir.dt.float32
F16 = mybir.dt.float16
I16 = mybir.dt.int16
OP = mybir.AluOpType
ACTF = mybir.ActivationFunctionType
AX = mybir.AxisListType

H = W = 512
P = 128
NBLK = H // P
MAGIC = float(2 ** 23)
N_CORES = 8
BPL = 4            # blocks per launch
NI = 1024          # idxs per dma_gather (ucode cap)
NGRP = 8           # groups per block (8 gathers / 64 out cols each)
NE16 = H * W // 8  # 32768 gather elements
NGS = 4            # rotating group-completion sems
ROWB = (W + 2) * 3  # staged row bytes-in-f32-elems (x and channel slack)


def _floor_exact(nc, pool, v, name):
    """floor of f32 tensor v via round-half-down magic (bias M - 0.5), no
    DVE fixup. At exact-integer ties v == k with k odd this returns k - 1
    instead of k, but the bilinear weights then put all mass on the x1 tap
    which IS k, so the sampled value is identical (verified against the
    fixed harness inputs)."""
    r = pool.tile([P, W], F32, name=f"fl_{name}")
    nc.scalar.activation(out=r, in_=v, func=ACTF.Copy, bias=MAGIC - 0.5)
    nc.scalar.activation(out=r, in_=r, func=ACTF.Copy, bias=-MAGIC)
    return r


def _body(ctx: ExitStack, tc: "tile.TileContext", imgs: bass.AP,
          theta: bass.AP, bb: bass.AP, gxr: bass.AP, pr: bass.AP,
          prs: bass.AP, out: bass.AP):
    nc = tc.nc

    imgQ = nc.dram_tensor("imgQ16", [NE16, 128], F16, kind="Internal").ap()

    stg_sem = nc.alloc_semaphore(name="stg_sem")
    gsems = [nc.alloc_semaphore(name=f"gs{i}") for i in range(NGS)]

    nc.gpsimd.load_library(library_config.mlp)
    nireg = nc.gpsimd.to_reg(NI)

    const_pool = ctx.enter_context(tc.tile_pool(name="const", bufs=1))
    stage_pool = ctx.enter_context(tc.tile_pool(name="stage", bufs=2))
    pairs_pool = ctx.enter_context(tc.tile_pool(name="pairs", bufs=2))
    coord_pool = ctx.enter_context(tc.tile_pool(name="coord", bufs=1))
    late_pool = ctx.enter_context(tc.tile_pool(name="late", bufs=2))
    tiny_pool = ctx.enter_context(tc.tile_pool(name="tiny", bufs=2))
    tt_pool = ctx.enter_context(tc.tile_pool(name="tt", bufs=2))
    idx_pool = ctx.enter_context(tc.tile_pool(name="idxp", bufs=BPL))
    pay_pool = ctx.enter_context(tc.tile_pool(name="pay", bufs=2))
    prod_pool = ctx.enter_context(tc.tile_pool(name="prod", bufs=1))
    quad_pool = ctx.enter_context(tc.tile_pool(name="quadm", bufs=1))
    outb_pool = ctx.enter_context(tc.tile_pool(name="outb", bufs=2))
    tabd_pool = ctx.enter_context(tc.tile_pool(name="tabd", bufs=BPL,
                                               space="DRAM"))

    # --- constants ---
    th = const_pool.tile([P, 6], F32)
    nc.sync.dma_start(out=th, in_=theta.unsqueeze(0).to_broadcast([P, 6]))
    bbs = const_pool.tile([P, BPL], F32)
    nc.sync.dma_start(out=bbs, in_=bb.unsqueeze(0).to_broadcast([P, BPL]))
    gx = const_pool.tile([P, W], F32)   # -1 + j*2/511 ramp on every partition
    nc.sync.dma_start(out=gx, in_=gxr.unsqueeze(0).to_broadcast([P, W]))
    pcol = const_pool.tile([P, 1], F32)  # partition index 0..127
    nc.sync.dma_start(out=pcol, in_=pr.unsqueeze(1))
    pcols = const_pool.tile([P, 1], F32)  # sigma(p) = (p%8)*16 + p//8
    nc.sync.dma_start(out=pcols, in_=prs.unsqueeze(1))

    a_ = th[:, 0:1]; b_ = th[:, 1:2]; c_ = th[:, 2:3]
    d_ = th[:, 3:4]; e_ = th[:, 4:5]; f_ = th[:, 5:6]

    # --- staging: build fp16 value-major element image in DRAM ---
    # element e = y*64 + k holds fp16[v*8 + s] = img[y+dy, 8k+s+dx, c]
    # with v = c*4 + (dy*2+dx); v/8 lanes for c==3 are junk (weight-0 side
    # reads of the next pixel's channel 0).
    imgs_flat = imgs.rearrange("k h w c -> k (h w c)")
    for blk in range(NBLK):
        r0 = blk * P
        loadAB = stage_pool.tile([P, 2, ROWB], F32)
        src = bass.AP(
            tensor=imgs_flat.tensor,
            offset=imgs_flat.offset + r0 * W * 3,
            ap=[[W * 3, P], [W * 3, 2], [1, ROWB]],
        )
        nc.scalar.dma_start(out=loadAB, in_=src)
        pairs = pairs_pool.tile([P, 64, 128], F16)
        ops = []
        for t in range(4):
            dy, dx = t >> 1, t & 1
            flat = loadAB[:, dy, :]
            # src: [c(4), k(64), s(8)] strides in f32 elems: c:1, k:24, s:3
            sap = bass.AP(tensor=flat.tensor, offset=flat.offset + dx * 3,
                          ap=[flat.ap[0], [1, 4], [24, 64], [3, 8]])
            # dst: pairs[p, k, (c*4+t)*8 + s] -> c:32, k:128, s:1
            dap = bass.AP(tensor=pairs.tensor, offset=pairs.offset + t * 8,
                          ap=[pairs.ap[0], [32, 4], [128, 64], [1, 8]])
            op = nc.scalar.activation(out=dap, in_=sap, func=ACTF.Copy)
            ops.append(op)
        if blk >= 1:
            # WAR vs the gpsimd stores of earlier pairs buffers
            for op in ops:
                op._wait_ge(stg_sem, 16 * blk)
        st = nc.gpsimd.dma_start(
            out=bass.AP(tensor=imgQ.tensor, offset=imgQ.offset + r0 * W * 16,
                        ap=[[W * 16, P], [1, W * 16]]),
            in_=pairs.rearrange("p w c -> p (w c)"))
        st.then_inc(stg_sem, 16)

    # --- per-output-block pipeline ---
    A256 = tiny_pool.tile([P, 1], F32, name="A256")
    nc.vector.tensor_scalar_mul(A256, a_, 256.0)
    D256 = tiny_pool.tile([P, 1], F32, name="D256")
    nc.vector.tensor_scalar_mul(D256, d_, 256.0)
    c1x = tiny_pool.tile([P, 1], F32, name="c1x")
    nc.vector.tensor_scalar(out=c1x, in0=c_, scalar1=1.0, scalar2=256.0,
                            op0=OP.add, op1=OP.mult)
    c1y = tiny_pool.tile([P, 1], F32, name="c1y")
    nc.vector.tensor_scalar(out=c1y, in0=f_, scalar1=1.0, scalar2=256.0,
                            op0=OP.add, op1=OP.mult)
    xa = tiny_pool.tile([P, W], F32, name="xa")
    nc.vector.tensor_scalar(out=xa, in0=gx, scalar1=A256, scalar2=None,
                            op0=OP.mult)
    ya = tiny_pool.tile([P, W], F32, name="ya")
    nc.vector.tensor_scalar(out=ya, in0=gx, scalar1=D256, scalar2=None,
                            op0=OP.mult)

    imgQ_ap = bass.AP(tensor=imgQ.tensor, offset=imgQ.offset,
                      ap=[[128, NE16], [1, 128]])

    def emit_coords(q):
        gyb = tiny_pool.tile([P, 1], F32, name="gyb")
        nc.vector.tensor_scalar(out=gyb, in0=pcol, scalar1=512.0 / 511.0,
                                scalar2=bbs[:, q:q + 1], op0=OP.mult,
                                op1=OP.add)
        sx = tiny_pool.tile([P, 1], F32, name="sx")
        nc.vector.tensor_scalar(out=sx, in0=gyb, scalar1=b_, scalar2=c1x,
                                op0=OP.mult, op1=OP.add)
        sy = tiny_pool.tile([P, 1], F32, name="sy")
        nc.vector.tensor_scalar(out=sy, in0=gyb, scalar1=e_, scalar2=c1y,
                                op0=OP.mult, op1=OP.add)

        def coord_side(arow, scol, tag):
            v = coord_pool.tile([P, W], F32, name=f"v{tag}")
            nc.vector.tensor_scalar(out=v, in0=arow, scalar1=scol,
                                    scalar2=None, op0=OP.add)
            # m = (v < 511): at exactly v == 511 the reference zeroes both
            # weights but half-down floor gives r=510/u1=1; mask u1 to 0.
            m = coord_pool.tile([P, W], F32, name="mside")
            nc.vector.tensor_scalar(out=m, in0=v, scalar1=511.0,
                                    scalar2=None, op0=OP.is_lt)
            r = _floor_exact(nc, coord_pool, v, tag)
            nc.vector.tensor_scalar(out=r, in0=r, scalar1=0.0, scalar2=511.0,
                                    op0=OP.max, op1=OP.min)
            p1 = coord_pool.tile([P, W], F32, name=f"p1{tag}")
            nc.vector.tensor_scalar(out=p1, in0=r, scalar1=1.0, scalar2=511.0,
                                    op0=OP.add, op1=OP.min)
            nc.vector.tensor_scalar(out=v, in0=v, scalar1=0.0, scalar2=511.0,
                                    op0=OP.max, op1=OP.min)
            nc.vector.tensor_sub(p1, p1, v)   # u0 = x1 - xc
            nc.vector.tensor_sub(v, v, r)     # u1 = xc - x0
            nc.vector.tensor_mul(v, v, m)     # kill u1 at v >= 511 exactly
            return p1, v, r

        u0, u1, x0f = coord_side(xa, sx, "x")
        v0, v1, y0f = coord_side(ya, sy, "y")

        # o = x0f & 7 (natural row order, for the mux one-hot)
        t8 = coord_pool.tile([P, W], F32, name="t8")
        nc.vector.tensor_scalar(out=t8, in0=x0f, scalar1=0.125,
                                scalar2=MAGIC, op0=OP.mult, op1=OP.add)
        nc.vector.tensor_scalar(out=t8, in0=t8, scalar1=-MAGIC, scalar2=None,
                                op0=OP.add)
        fx = coord_pool.tile([P, W], F32, name="fx")
        nc.vector.scalar_tensor_tensor(out=fx, in0=t8, scalar=8.0,
                                       in1=x0f, op0=OP.mult, op1=OP.is_gt)
        nc.vector.tensor_sub(t8, t8, fx)      # t8 = xq
        o = coord_pool.tile([P, W], F32, name="fx")  # reuse fx's buffer
        nc.vector.scalar_tensor_tensor(out=o, in0=t8, scalar=-8.0,
                                       in1=x0f, op0=OP.mult, op1=OP.add)

        # one-hot of o: oh[p, j, s] = (o == s), fp16, s packed innermost
        oh = late_pool.tile([P, W, 8], F16, name="oh")
        for s in range(8):
            nc.vector.tensor_scalar(out=oh[:, :, s], in0=o,
                                    scalar1=float(s), scalar2=None,
                                    op0=OP.is_equal)

        # W4[p, j, t] = bilinear tap weights (fp16): t0=u0v0 t1=u1v0
        # t2=u0v1 t3=u1v1
        w4 = late_pool.tile([P, W, 4], F16, name="w4")
        nc.vector.tensor_mul(w4[:, :, 0], u0, v0)
        nc.vector.tensor_mul(w4[:, :, 1], u1, v0)
        nc.vector.tensor_mul(w4[:, :, 2], u0, v1)
        nc.vector.tensor_mul(w4[:, :, 3], u1, v1)

        # idx in NATURAL row order from already-computed y0f/t8:
        # idx = y0 * 64 + (x0 >> 3)
        idxf = coord_pool.tile([P, W], F32, name="idxf")
        nc.vector.scalar_tensor_tensor(out=idxf, in0=y0f, scalar=64.0,
                                       in1=t8, op0=OP.mult, op1=OP.add)
        idx16 = coord_pool.tile([P, W], I16, name="idx16")
        nc.vector.tensor_copy(out=idx16, in_=idxf)

        # --- idx shuffle into wrapped layout via transpose + DRAM trip ---
        # tT[a, c*128+b] = idx of pixel (row b, col c*128+a). The wrapped
        # table needs tD[q][j*8+u] = idx of pixel (16u+q, j); reading tT's
        # free dim as b = i1 + 16*i2 (strides [1,16],[16,8]) delivers rows
        # in exactly that order, so no sigma-ordered second coord pass.
        tD = tabd_pool.tile([16, W * 8], I16, name="tabD")
        tT = tt_pool.tile([P, W], I16, name="tT")
        for c in range(W // P):
            nc.sync.dma_start_transpose(
                out=tT[:, c * P:(c + 1) * P],
                in_=idx16[:, c * P:(c + 1) * P])
        for c in range(W // P):
            nc.scalar.dma_start(
                out=bass.AP(tensor=tD.tensor, offset=tD.offset + c * P * 8,
                            ap=[[8, P], [1, 8], [W * 8, 16]]),
                in_=tT[:, c * P:(c + 1) * P])
        # idx bufs == BPL so there is no buffer reuse within a launch; the
        # tile framework tracks this SP DMA write -> gather-prep read dep.
        idx_sb = idx_pool.tile([P, W * 8], I16, name="idxsb")
        nc.sync.dma_start(
            out=idx_sb,
            in_=bass.AP(tensor=tD.tensor, offset=tD.offset,
                        ap=[[0, 8], [W * 8, 16], [1, W * 8]]))
        return oh, w4, idx_sb

    def emit_mux(q, oh, w4, idx_sb):
        # --- gathers + fused mux + blend per 64-col group ---
        outblk = outb_pool.tile([P, W, 3], F32, name="outblk")
        for grp in range(NGRP):
            gg = q * NGRP + grp  # global group id
            gsem = gsems[gg % NGS]
            pay = pay_pool.tile([P, 64, 128], F16, name="pay")
            for l in range(8):
                g = grp * 8 + l
                prep = nc.gpsimd.dma_gather(
                    out_ap=pay[:, l * 8:(l + 1) * 8, :],
                    in_ap=imgQ_ap,
                    idxs_ap=idx_sb[:, g * 64:(g + 1) * 64],
                    num_idxs=NI,
                    num_idxs_reg=nireg,
                    elem_size=128,
                    prepare_only=True,
                    sem=gsem,
                )
                trig = nc.gpsimd.trigger_dma(count=None)
                if gg == 0 and l == 0:
                    trig._wait_ge(stg_sem, 16 * NBLK)
                elif gg >= 2 and l == 0:
                    pg = gg - 2
                    trig._wait_ge(gsems[pg % NGS], 128 * (pg // NGS + 1))
            sl = slice(grp * 64, (grp + 1) * 64)
            # fused mux: prod[p, j, v, s] = pay * oh (fp16 2x), then add
            # tree over s: 8 -> 4 -> 2 -> 1
            prod = prod_pool.tile([P, 64, 12, 8], F16, name="prod")
            payv = bass.AP(tensor=pay.tensor, offset=pay.offset,
                           ap=[pay.ap[0], [128, 64], [8, 12], [1, 8]])
            ohv = bass.AP(tensor=oh.tensor, offset=oh.offset + grp * 64 * 8,
                          ap=[oh.ap[0], [8, 64], [0, 12], [1, 8]])
            pm = nc.vector.tensor_mul(prod, payv, ohv)
            pm._wait_ge(gsem, 128 * (gg // NGS + 1))
            # add tree in-place inside prod: sum4 -> prod[...,0:4],
            # sum2 -> prod[...,4:6]
            p0 = bass.AP(tensor=prod.tensor, offset=prod.offset,
                         ap=[prod.ap[0], [96, 64], [8, 12], [1, 4]])
            p1 = bass.AP(tensor=prod.tensor, offset=prod.offset + 4,
                         ap=[prod.ap[0], [96, 64], [8, 12], [1, 4]])
            nc.vector.tensor_add(p0, p0, p1)
            s40 = bass.AP(tensor=prod.tensor, offset=prod.offset,
                          ap=[prod.ap[0], [96, 64], [8, 12], [1, 2]])
            s41 = bass.AP(tensor=prod.tensor, offset=prod.offset + 2,
                          ap=[prod.ap[0], [96, 64], [8, 12], [1, 2]])
            s2o = bass.AP(tensor=prod.tensor, offset=prod.offset + 4,
                          ap=[prod.ap[0], [96, 64], [8, 12], [1, 2]])
            nc.vector.tensor_add(s2o, s40, s41)
            quadm = quad_pool.tile([P, 64, 12], F16, name="quadm")
            s20 = bass.AP(tensor=prod.tensor, offset=prod.offset + 4,
                          ap=[prod.ap[0], [96, 64], [8, 12]])
            s21 = bass.AP(tensor=prod.tensor, offset=prod.offset + 5,
                          ap=[prod.ap[0], [96, 64], [8, 12]])
            nc.vector.tensor_add(quadm, s20, s21)
            # blend: bmul[p, j, c, t] = quadm * W4, add tree 4 -> 2 -> 1
            bmul = quad_pool.tile([P, 64, 3, 4], F16, name="bmul")
            qv = bass.AP(tensor=quadm.tensor, offset=quadm.offset,
                         ap=[quadm.ap[0], [12, 64], [4, 3], [1, 4]])
            wv = bass.AP(tensor=w4.tensor, offset=w4.offset + grp * 64 * 4,
                         ap=[w4.ap[0], [4, 64], [0, 3], [1, 4]])
            nc.vector.tensor_mul(bmul, qv, wv)
            b2 = quad_pool.tile([P, 64, 3, 2], F16, name="b2")
            b20 = bass.AP(tensor=bmul.tensor, offset=bmul.offset,
                          ap=[bmul.ap[0], [12, 64], [4, 3], [1, 2]])
            b21 = bass.AP(tensor=bmul.tensor, offset=bmul.offset + 2,
                          ap=[bmul.ap[0], [12, 64], [4, 3], [1, 2]])
            nc.vector.tensor_add(b2, b20, b21)
            ob = bass.AP(tensor=outblk.tensor,
                         offset=outblk.offset + grp * 64 * 3,
                         ap=[outblk.ap[0], [3, 64], [1, 3]])
            b2e = bass.AP(tensor=b2.tensor, offset=b2.offset,
                          ap=[b2.ap[0], [6, 64], [2, 3]])
            b2o = bass.AP(tensor=b2.tensor, offset=b2.offset + 1,
                          ap=[b2.ap[0], [6, 64], [2, 3]])
            nc.vector.tensor_add(ob, b2e, b2o)
        nc.sync.dma_start(out=out[q], in_=outblk)

    # software pipeline: coords/idx of block q+1 are emitted before block
    # q's muxes so the DVE never stalls at block boundaries.
    blkdata = {0: emit_coords(0)}
    for q in range(BPL):
        if q + 1 < BPL:
            blkdata[q + 1] = emit_coords(q + 1)
        emit_mux(q, *blkdata.pop(q))


def build_kernel2(num_devices: int = N_CORES):
    nc = bacc.Bacc("TRN2", target_bir_lowering=False, debug=False,
                   num_devices=num_devices)
    imgs = nc.dram_tensor("imgs", [1, H + 2, W, 3], F32, kind="ExternalInput")
    theta = nc.dram_tensor("theta", [6], F32, kind="ExternalInput")
    bb = nc.dram_tensor("bb", [BPL], F32, kind="ExternalInput")
    gxr = nc.dram_tensor("gxr", [W], F32, kind="ExternalInput")
    pr = nc.dram_tensor("pr", [P], F32, kind="ExternalInput")
    prs = nc.dram_tensor("prs", [P], F32, kind="ExternalInput")
    out = nc.dram_tensor("out", [BPL, P, W, 3], F32, kind="ExternalOutput")
    with tile.TileContext(nc) as tc:
        with ExitStack() as ctx:
            _body(ctx, tc, imgs.ap(), theta.ap(), bb.ap(), gxr.ap(), pr.ap(),
                  prs.ap(), out.ap())
    nc.compile()
    return nc


_NC_CACHE = {}


def run_kernel_spmd(images: np.ndarray, theta: np.ndarray, trace: bool = False):
    B = images.shape[0]
    per = B // N_CORES
    if "k3" not in _NC_CACHE:
        _NC_CACHE["k3"] = build_kernel2(N_CORES)
    nc = _NC_CACHE["k3"]

    out = np.zeros((B, H, W, 3), np.float32)
    slabs = []
    for c in range(N_CORES):
        s = np.zeros((per, H + 2, W, 3), np.float32)
        s[:, :H] = images[c * per:(c + 1) * per]
        slabs.append(s)

    gxr = (np.arange(W, dtype=np.float32) * (2.0 / 511.0) - 1.0).astype(
        np.float32)
    prv = np.arange(P, dtype=np.float32)
    pi = np.arange(P)
    prsv = ((pi % 8) * 16 + pi // 8).astype(np.float32)
    bbv = np.array([128.0 * q * (512.0 / 511.0) - 256.0 for q in range(BPL)],
                   np.float32)

    last_res = None
    for k in range(per):
        in_maps = []
        for c in range(N_CORES):
            in_maps.append({
                "imgs": slabs[c][k:k + 1],
                "theta": np.ascontiguousarray(
                    theta[c * per + k].reshape(-1)).astype(np.float32),
                "bb": bbv,
                "gxr": gxr,
                "pr": prv,
                "prs": prsv,
            })
        res = run_bass_kernel_spmd(nc, in_maps, core_ids=list(range(N_CORES)),
                                   trace=trace)
        last_res = res
        for c in range(N_CORES):
            out[c * per + k] = res.results[c]["out"].reshape(H, W, 3)
    return out, last_res


def kernel(images: np.ndarray, theta: np.ndarray) -> np.ndarray:
    images = np.ascontiguousarray(np.asarray(images), dtype=np.float32)
    theta = np.asarray(theta).astype(np.float32)
    out, _ = run_kernel_spmd(images, theta, trace=False)
    return out


# revision 57
# speedup vs baseline: 1.8150x; 1.0440x over previous
"""Trainium2 Bass kernel v3: batched affine bilinear sampling via dma_gather.

Full inputs: images [32, 512, 512, 3] f32, theta [32, 2, 3] f32.
Data parallel over batch: 8 NeuronCores x 4 images; one SPMD launch per image
(4 output blocks of 128 rows per launch).

v3 vs v2: the gather element is VALUE-MAJOR so the slot dim (s = x&7) is
packed innermost: element for (y, octet k) holds 128 fp16 at position
v*8 + s with v = c*4 + tap, tap = dy*2+dx (c=3 lane is junk padding).
This lets the 8-way slot mux run as ONE fused fp16 tensor_tensor multiply
per 64-column group (DVE 2x mode) followed by a packed pairwise add tree,
and the bilinear blend as a second fused multiply + small add tree.
DMA issue for staging/idx/output moved off the Pool engine (gather preps
own it).

Device algorithm per launch:
  1. Stage the fp16 value-major element image in DRAM: 32768 elements of
     256B; element index = y*64 + (x>>3) <= 32767 fits int16.
  2. Per 128-row output block: compute sample coords / lerp weights, the
     element index idx = y0*64 + (x0>>3), slot o = x0 & 7, onehot(o) and
     the 4 bilinear tap weights W4 = [u0v0, u1v0, u0v1, u1v1] (fp16).
  3. Shuffle idx into dma_gather's wrapped index layout via int16
     DMA-transpose + a strided DRAM round trip, broadcast to all 8 gpsimd
     index stripes.
  4. 64 dma_gathers per block (1024 idxs each - ucode cap), prepare_only +
     trigger, 4 rotating per-group completion sems (2 groups in flight).
  5. Per group: prod = pay * onehot (fp16 2x), add tree 8->4->2->1 to mux
     the right slot, then blend via quadm * W4 + add tree 4->2->1 -> f32.
"""

import sys
from contextlib import ExitStack

for _p in ("/opt/trn_rl_repo",):
    if _p not in sys.path:
        sys.path.append(_p)

import numpy as np

import concourse.bacc as bacc
import concourse.bass as bass
import concourse.tile as tile
from concourse import library_config, mybir
from concourse.bass_utils import run_bass_kernel_spmd

F32 = mybir.dt.float32
F16 = mybir.dt.float16
I16 = mybir.dt.int16
OP = mybir.AluOpType
ACTF = mybir.ActivationFunctionType
AX = mybir.AxisListType

H = W = 512
P = 128
NBLK = H // P
MAGIC = float(2 ** 23)
N_CORES = 8
BPL = 4            # blocks per launch
NI = 1024          # idxs per dma_gather (ucode cap)
NGRP = 8           # groups per block (8 gathers / 64 out cols each)
NE16 = H * W // 8  # 32768 gather elements
NGS = 4            # rotating group-completion sems
ROWB = (W + 2) * 3  # staged row bytes-in-f32-elems (x and channel slack)


def _floor_exact(nc, pool, v, name):
    """floor of f32 tensor v via round-half-down magic (bias M - 0.5), no
    DVE fixup. At exact-integer ties v == k with k odd this returns k - 1
    instead of k, but the bilinear weights then put all mass on the x1 tap
    which IS k, so the sampled value is identical (verified against the
    fixed harness inputs)."""
    r = pool.tile([P, W], F32, name=f"fl_{name}")
    nc.scalar.activation(out=r, in_=v, func=ACTF.Copy, bias=MAGIC - 0.5)
    nc.scalar.activation(out=r, in_=r, func=ACTF.Copy, bias=-MAGIC)
    return r


def _body(ctx: ExitStack, tc: "tile.TileContext", imgs: bass.AP,
          theta: bass.AP, bb: bass.AP, gxr: bass.AP, pr: bass.AP,
          prs: bass.AP, out: bass.AP):
    nc = tc.nc

    imgQ = nc.dram_tensor("imgQ16", [NE16, 128], F16, kind="Internal").ap()

    stg_sem = nc.alloc_semaphore(name="stg_sem")
    gsems = [nc.alloc_semaphore(name=f"gs{i}") for i in range(NGS)]

    nc.gpsimd.load_library(library_config.mlp)
    nireg = nc.gpsimd.to_reg(NI)

    const_pool = ctx.enter_context(tc.tile_pool(name="const", bufs=1))
    stage_pool = ctx.enter_context(tc.tile_pool(name="stage", bufs=2))
    pairs_pool = ctx.enter_context(tc.tile_pool(name="pairs", bufs=2))
    coord_pool = ctx.enter_context(tc.tile_pool(name="coord", bufs=1))
    late_pool = ctx.enter_context(tc.tile_pool(name="late", bufs=2))
    tiny_pool = ctx.enter_context(tc.tile_pool(name="tiny", bufs=2))
    tt_pool = ctx.enter_context(tc.tile_pool(name="tt", bufs=2))
    idx_pool = ctx.enter_context(tc.tile_pool(name="idxp", bufs=BPL))
    pay_pool = ctx.enter_context(tc.tile_pool(name="pay", bufs=2))
    prod_pool = ctx.enter_context(tc.tile_pool(name="prod", bufs=1))
    quad_pool = ctx.enter_context(tc.tile_pool(name="quadm", bufs=1))
    outb_pool = ctx.enter_context(tc.tile_pool(name="outb", bufs=2))
    tabd_pool = ctx.enter_context(tc.tile_pool(name="tabd", bufs=BPL,
                                               space="DRAM"))

    # --- constants ---
    th = const_pool.tile([P, 6], F32)
    nc.sync.dma_start(out=th, in_=theta.unsqueeze(0).to_broadcast([P, 6]))
    bbs = const_pool.tile([P, BPL], F32)
    nc.sync.dma_start(out=bbs, in_=bb.unsqueeze(0).to_broadcast([P, BPL]))
    gx = const_pool.tile([P, W], F32)   # -1 + j*2/511 ramp on every partition
    nc.sync.dma_start(out=gx, in_=gxr.unsqueeze(0).to_broadcast([P, W]))
    pcol = const_pool.tile([P, 1], F32)  # partition index 0..127
    nc.sync.dma_start(out=pcol, in_=pr.unsqueeze(1))
    pcols = const_pool.tile([P, 1], F32)  # sigma(p) = (p%8)*16 + p//8
    nc.sync.dma_start(out=pcols, in_=prs.unsqueeze(1))

    a_ = th[:, 0:1]; b_ = th[:, 1:2]; c_ = th[:, 2:3]
    d_ = th[:, 3:4]; e_ = th[:, 4:5]; f_ = th[:, 5:6]

    # --- staging: build fp16 value-major element image in DRAM ---
    # element e = y*64 + k holds fp16[v*8 + s] = img[y+dy, 8k+s+dx, c]
    # with v = c*4 + (dy*2+dx); v/8 lanes for c==3 are junk (weight-0 side
    # reads of the next pixel's channel 0). Emitted AFTER the first two
    # blocks' coord work so Act floor ops are not queued behind the copies.
    imgs_flat = imgs.rearrange("k h w c -> k (h w c)")

    def emit_staging_blk(blk):
        r0 = blk * P
        loadAB = stage_pool.tile([P, 2, ROWB], F32)
        src = bass.AP(
            tensor=imgs_flat.tensor,
            offset=imgs_flat.offset + r0 * W * 3,
            ap=[[W * 3, P], [W * 3, 2], [1, ROWB]],
        )
        nc.scalar.dma_start(out=loadAB, in_=src)
        pairs = pairs_pool.tile([P, 64, 128], F16)
        ops = []
        for t in range(4):
            dy, dx = t >> 1, t & 1
            flat = loadAB[:, dy, :]
            # src: [c(4), k(64), s(8)] strides in f32 elems: c:1, k:24, s:3
            sap = bass.AP(tensor=flat.tensor, offset=flat.offset + dx * 3,
                          ap=[flat.ap[0], [1, 4], [24, 64], [3, 8]])
            # dst: pairs[p, k, (c*4+t)*8 + s] -> c:32, k:128, s:1
            dap = bass.AP(tensor=pairs.tensor, offset=pairs.offset + t * 8,
                          ap=[pairs.ap[0], [32, 4], [128, 64], [1, 8]])
            if t < 2:
                op = nc.vector.tensor_copy(out=dap, in_=sap)
            else:
                op = nc.scalar.activation(out=dap, in_=sap, func=ACTF.Copy)
            ops.append(op)
        # SP-queue store; the framework tracks its read of `pairs`, so the
        # next round's copies auto-wait. Completion ordering vs the token
        # below comes from same-queue FIFO.
        nc.sync.dma_start(
            out=bass.AP(tensor=imgQ.tensor, offset=imgQ.offset + r0 * W * 16,
                        ap=[[W * 16, P], [1, W * 16]]),
            in_=pairs.rearrange("p w c -> p (w c)"))

    # --- per-output-block pipeline ---
    A256 = tiny_pool.tile([P, 1], F32, name="A256")
    nc.vector.tensor_scalar_mul(A256, a_, 256.0)
    D256 = tiny_pool.tile([P, 1], F32, name="D256")
    nc.vector.tensor_scalar_mul(D256, d_, 256.0)
    c1x = tiny_pool.tile([P, 1], F32, name="c1x")
    nc.vector.tensor_scalar(out=c1x, in0=c_, scalar1=1.0, scalar2=256.0,
                            op0=OP.add, op1=OP.mult)
    c1y = tiny_pool.tile([P, 1], F32, name="c1y")
    nc.vector.tensor_scalar(out=c1y, in0=f_, scalar1=1.0, scalar2=256.0,
                            op0=OP.add, op1=OP.mult)
    xa = tiny_pool.tile([P, W], F32, name="xa")
    nc.vector.tensor_scalar(out=xa, in0=gx, scalar1=A256, scalar2=None,
                            op0=OP.mult)
    ya = tiny_pool.tile([P, W], F32, name="ya")
    nc.vector.tensor_scalar(out=ya, in0=gx, scalar1=D256, scalar2=None,
                            op0=OP.mult)

    imgQ_ap = bass.AP(tensor=imgQ.tensor, offset=imgQ.offset,
                      ap=[[128, NE16], [1, 128]])

    def emit_coords(q):
        gyb = tiny_pool.tile([P, 1], F32, name="gyb")
        nc.vector.tensor_scalar(out=gyb, in0=pcol, scalar1=512.0 / 511.0,
                                scalar2=bbs[:, q:q + 1], op0=OP.mult,
                                op1=OP.add)
        sx = tiny_pool.tile([P, 1], F32, name="sx")
        nc.vector.tensor_scalar(out=sx, in0=gyb, scalar1=b_, scalar2=c1x,
                                op0=OP.mult, op1=OP.add)
        sy = tiny_pool.tile([P, 1], F32, name="sy")
        nc.vector.tensor_scalar(out=sy, in0=gyb, scalar1=e_, scalar2=c1y,
                                op0=OP.mult, op1=OP.add)

        def coord_side(arow, scol, tag):
            v = coord_pool.tile([P, W], F32, name=f"v{tag}")
            nc.vector.tensor_scalar(out=v, in0=arow, scalar1=scol,
                                    scalar2=None, op0=OP.add)
            # m = (v < 511): at exactly v == 511 the reference zeroes both
            # weights but half-down floor gives r=510/u1=1; mask u1 to 0.
            m = coord_pool.tile([P, W], F32, name="mside")
            nc.vector.tensor_scalar(out=m, in0=v, scalar1=511.0,
                                    scalar2=None, op0=OP.is_lt)
            r = _floor_exact(nc, coord_pool, v, tag)
            nc.vector.tensor_scalar(out=r, in0=r, scalar1=0.0, scalar2=511.0,
                                    op0=OP.max, op1=OP.min)
            p1 = coord_pool.tile([P, W], F32, name=f"p1{tag}")
            nc.vector.tensor_scalar(out=p1, in0=r, scalar1=1.0, scalar2=511.0,
                                    op0=OP.add, op1=OP.min)
            nc.vector.tensor_scalar(out=v, in0=v, scalar1=0.0, scalar2=511.0,
                                    op0=OP.max, op1=OP.min)
            nc.vector.tensor_sub(p1, p1, v)   # u0 = x1 - xc
            nc.vector.tensor_sub(v, v, r)     # u1 = xc - x0
            nc.vector.tensor_mul(v, v, m)     # kill u1 at v >= 511 exactly
            return p1, v, r

        u0, u1, x0f = coord_side(xa, sx, "x")
        v0, v1, y0f = coord_side(ya, sy, "y")

        # o = x0f & 7 (natural row order, for the mux one-hot)
        t8 = coord_pool.tile([P, W], F32, name="t8")
        nc.vector.tensor_scalar(out=t8, in0=x0f, scalar1=0.125,
                                scalar2=MAGIC, op0=OP.mult, op1=OP.add)
        nc.vector.tensor_scalar(out=t8, in0=t8, scalar1=-MAGIC, scalar2=None,
                                op0=OP.add)
        fx = coord_pool.tile([P, W], F32, name="fx")
        nc.vector.scalar_tensor_tensor(out=fx, in0=t8, scalar=8.0,
                                       in1=x0f, op0=OP.mult, op1=OP.is_gt)
        nc.vector.tensor_sub(t8, t8, fx)      # t8 = xq
        o = coord_pool.tile([P, W], F32, name="fx")  # reuse fx's buffer
        nc.vector.scalar_tensor_tensor(out=o, in0=t8, scalar=-8.0,
                                       in1=x0f, op0=OP.mult, op1=OP.add)

        # one-hot of o: oh[p, j, s] = (o == s), fp16, s packed innermost
        oh = late_pool.tile([P, W, 8], F16, name="oh")
        for s in range(8):
            nc.vector.tensor_scalar(out=oh[:, :, s], in0=o,
                                    scalar1=float(s), scalar2=None,
                                    op0=OP.is_equal)

        # W4[p, j, t] = bilinear tap weights (fp16): t0=u0v0 t1=u1v0
        # t2=u0v1 t3=u1v1
        w4 = late_pool.tile([P, W, 4], F16, name="w4")
        nc.vector.tensor_mul(w4[:, :, 0], u0, v0)
        nc.vector.tensor_mul(w4[:, :, 1], u1, v0)
        nc.vector.tensor_mul(w4[:, :, 2], u0, v1)
        nc.vector.tensor_mul(w4[:, :, 3], u1, v1)

        # idx in NATURAL row order from already-computed y0f/t8:
        # idx = y0 * 64 + (x0 >> 3)
        idxf = coord_pool.tile([P, W], F32, name="idxf")
        nc.vector.scalar_tensor_tensor(out=idxf, in0=y0f, scalar=64.0,
                                       in1=t8, op0=OP.mult, op1=OP.add)
        idx16 = coord_pool.tile([P, W], I16, name="idx16")
        nc.vector.tensor_copy(out=idx16, in_=idxf)

        # --- idx shuffle into wrapped layout via transpose + DRAM trip ---
        # tT[a, c*128+b] = idx of pixel (row b, col c*128+a). The wrapped
        # table needs tD[q][j*8+u] = idx of pixel (16u+q, j); reading tT's
        # free dim as b = i1 + 16*i2 (strides [1,16],[16,8]) delivers rows
        # in exactly that order, so no sigma-ordered second coord pass.
        tD = tabd_pool.tile([16, W * 8], I16, name="tabD")
        tT = tt_pool.tile([P, W], I16, name="tT")
        for c in range(W // P):
            nc.sync.dma_start_transpose(
                out=tT[:, c * P:(c + 1) * P],
                in_=idx16[:, c * P:(c + 1) * P])
        for c in range(W // P):
            nc.scalar.dma_start(
                out=bass.AP(tensor=tD.tensor, offset=tD.offset + c * P * 8,
                            ap=[[8, P], [1, 8], [W * 8, 16]]),
                in_=tT[:, c * P:(c + 1) * P])
        # idx bufs == BPL so there is no buffer reuse within a launch; the
        # tile framework tracks this SP DMA write -> gather-prep read dep.
        idx_sb = idx_pool.tile([P, W * 8], I16, name="idxsb")
        nc.sync.dma_start(
            out=idx_sb,
            in_=bass.AP(tensor=tD.tensor, offset=tD.offset,
                        ap=[[0, 8], [W * 8, 16], [1, W * 8]]))
        return oh, w4, idx_sb

    def emit_mux(q, oh, w4, idx_sb):
        # --- gathers + fused mux + blend per 64-col group ---
        outblk = outb_pool.tile([P, W, 3], F32, name="outblk")
        for grp in range(NGRP):
            gg = q * NGRP + grp  # global group id
            gsem = gsems[gg % NGS]
            pay = pay_pool.tile([P, 64, 128], F16, name="pay")
            for l in range(8):
                g = grp * 8 + l
                prep = nc.gpsimd.dma_gather(
                    out_ap=pay[:, l * 8:(l + 1) * 8, :],
                    in_ap=imgQ_ap,
                    idxs_ap=idx_sb[:, g * 64:(g + 1) * 64],
                    num_idxs=NI,
                    num_idxs_reg=nireg,
                    elem_size=128,
                    prepare_only=True,
                    sem=gsem,
                )
                trig = nc.gpsimd.trigger_dma(count=None)
                if gg == 0 and l == 0:
                    trig._wait_ge(stg_sem, 16 * NBLK)
                elif gg >= 2 and l == 0:
                    pg = gg - 2
                    trig._wait_ge(gsems[pg % NGS], 128 * (pg // NGS + 1))
            sl = slice(grp * 64, (grp + 1) * 64)
            # fused mux: prod[p, j, v, s] = pay * oh (fp16 2x), then add
            # tree over s: 8 -> 4 -> 2 -> 1
            prod = prod_pool.tile([P, 64, 12, 8], F16, name="prod")
            payv = bass.AP(tensor=pay.tensor, offset=pay.offset,
                           ap=[pay.ap[0], [128, 64], [8, 12], [1, 8]])
            ohv = bass.AP(tensor=oh.tensor, offset=oh.offset + grp * 64 * 8,
                          ap=[oh.ap[0], [8, 64], [0, 12], [1, 8]])
            pm = nc.vector.tensor_mul(prod, payv, ohv)
            pm._wait_ge(gsem, 128 * (gg // NGS + 1))
            # add tree in-place inside prod: sum4 -> prod[...,0:4],
            # sum2 -> prod[...,4:6]
            p0 = bass.AP(tensor=prod.tensor, offset=prod.offset,
                         ap=[prod.ap[0], [96, 64], [8, 12], [1, 4]])
            p1 = bass.AP(tensor=prod.tensor, offset=prod.offset + 4,
                         ap=[prod.ap[0], [96, 64], [8, 12], [1, 4]])
            nc.vector.tensor_add(p0, p0, p1)
            s40 = bass.AP(tensor=prod.tensor, offset=prod.offset,
                          ap=[prod.ap[0], [96, 64], [8, 12], [1, 2]])
            s41 = bass.AP(tensor=prod.tensor, offset=prod.offset + 2,
                          ap=[prod.ap[0], [96, 64], [8, 12], [1, 2]])
            s2o = bass.AP(tensor=prod.tensor, offset=prod.offset + 4,
                          ap=[prod.ap[0], [96, 64], [8, 12], [1, 2]])
            nc.vector.tensor_add(s2o, s40, s41)
            quadm = quad_pool.tile([P, 64, 12], F16, name="quadm")
            s20 = bass.AP(tensor=prod.tensor, offset=prod.offset + 4,
                          ap=[prod.ap[0], [96, 64], [8, 12]])
            s21 = bass.AP(tensor=prod.tensor, offset=prod.offset + 5,
                          ap=[prod.ap[0], [96, 64], [8, 12]])
            nc.vector.tensor_add(quadm, s20, s21)
            # blend: bmul[p, j, c, t] = quadm * W4, add tree 4 -> 2 -> 1
            bmul = quad_pool.tile([P, 64, 3, 4], F16, name="bmul")
            qv = bass.AP(tensor=quadm.tensor, offset=quadm.offset,
                         ap=[quadm.ap[0], [12, 64], [4, 3], [1, 4]])
            wv = bass.AP(tensor=w4.tensor, offset=w4.offset + grp * 64 * 4,
                         ap=[w4.ap[0], [4, 64], [0, 3], [1, 4]])
            nc.vector.tensor_mul(bmul, qv, wv)
            b2 = quad_pool.tile([P, 64, 3, 2], F16, name="b2")
            b20 = bass.AP(tensor=bmul.tensor, offset=bmul.offset,
                          ap=[bmul.ap[0], [12, 64], [4, 3], [1, 2]])
            b21 = bass.AP(tensor=bmul.tensor, offset=bmul.offset + 2,
                          ap=[bmul.ap[0], [12, 64], [4, 3], [1, 2]])
            nc.vector.tensor_add(b2, b20, b21)
            ob = bass.AP(tensor=outblk.tensor,
                         offset=outblk.offset + grp * 64 * 3,
                         ap=[outblk.ap[0], [3, 64], [1, 3]])
            b2e = bass.AP(tensor=b2.tensor, offset=b2.offset,
                          ap=[b2.ap[0], [6, 64], [2, 3]])
            b2o = bass.AP(tensor=b2.tensor, offset=b2.offset + 1,
                          ap=[b2.ap[0], [6, 64], [2, 3]])
            nc.vector.tensor_add(ob, b2e, b2o)
        nc.sync.dma_start(out=out[q], in_=outblk)

    # software pipeline: coords/idx of blocks 0 and 1 come first (filling
    # the staging shadow), then staging, then mux(q) alternating with
    # coords(q+2) so the DVE never stalls at block boundaries.
    emit_staging_blk(0)
    emit_staging_blk(1)
    _early_coords = emit_coords(0)
    emit_staging_blk(2)
    emit_staging_blk(3)
    # Token on the same SP queue: completes after all 4 imgQ stores
    # (per-queue FIFO). HWDGE DMAs can't carry manual sems, so a tiny Pool
    # (SWDGE) echo DMA reads the token tile (framework-ordered after the
    # token lands) and carries the staging-done semaphore.
    scrapD = nc.dram_tensor("scrapD", [1], F32, kind="Internal")
    tok_t = const_pool.tile([1, 1], F32, name="tok")
    nc.sync.dma_start(out=tok_t, in_=bb[0:1].unsqueeze(0))
    echo = nc.gpsimd.dma_start(out=scrapD.ap().unsqueeze(0), in_=tok_t)
    echo.then_inc(stg_sem, 16 * NBLK)
    blkdata = {0: _early_coords}
    for q in range(BPL):
        if q + 1 < BPL:
            blkdata[q + 1] = emit_coords(q + 1)
        emit_mux(q, *blkdata.pop(q))


def build_kernel2(num_devices: int = N_CORES):
    nc = bacc.Bacc("TRN2", target_bir_lowering=False, debug=False,
                   num_devices=num_devices)
    imgs = nc.dram_tensor("imgs", [1, H + 2, W, 3], F32, kind="ExternalInput")
    theta = nc.dram_tensor("theta", [6], F32, kind="ExternalInput")
    bb = nc.dram_tensor("bb", [BPL], F32, kind="ExternalInput")
    gxr = nc.dram_tensor("gxr", [W], F32, kind="ExternalInput")
    pr = nc.dram_tensor("pr", [P], F32, kind="ExternalInput")
    prs = nc.dram_tensor("prs", [P], F32, kind="ExternalInput")
    out = nc.dram_tensor("out", [BPL, P, W, 3], F32, kind="ExternalOutput")
    with tile.TileContext(nc) as tc:
        with ExitStack() as ctx:
            _body(ctx, tc, imgs.ap(), theta.ap(), bb.ap(), gxr.ap(), pr.ap(),
                  prs.ap(), out.ap())
    nc.compile()
    return nc


_NC_CACHE = {}


def run_kernel_spmd(images: np.ndarray, theta: np.ndarray, trace: bool = False):
    B = images.shape[0]
    per = B // N_CORES
    if "k3" not in _NC_CACHE:
        _NC_CACHE["k3"] = build_kernel2(N_CORES)
    nc = _NC_CACHE["k3"]

    out = np.zeros((B, H, W, 3), np.float32)
    slabs = []
    for c in range(N_CORES):
        s = np.zeros((per, H + 2, W, 3), np.float32)
        s[:, :H] = images[c * per:(c + 1) * per]
        slabs.append(s)

    gxr = (np.arange(W, dtype=np.float32) * (2.0 / 511.0) - 1.0).astype(
        np.float32)
    prv = np.arange(P, dtype=np.float32)
    pi = np.arange(P)
    prsv = ((pi % 8) * 16 + pi // 8).astype(np.float32)
    bbv = np.array([128.0 * q * (512.0 / 511.0) - 256.0 for q in range(BPL)],
                   np.float32)

    last_res = None
    for k in range(per):
        in_maps = []
        for c in range(N_CORES):
            in_maps.append({
                "imgs": slabs[c][k:k + 1],
                "theta": np.ascontiguousarray(
                    theta[c * per + k].reshape(-1)).astype(np.float32),
                "bb": bbv,
                "gxr": gxr,
                "pr": prv,
                "prs": prsv,
            })
        res = run_bass_kernel_spmd(nc, in_maps, core_ids=list(range(N_CORES)),
                                   trace=trace)
        last_res = res
        for c in range(N_CORES):
            out[c * per + k] = res.results[c]["out"].reshape(H, W, 3)
    return out, last_res


def kernel(images: np.ndarray, theta: np.ndarray) -> np.ndarray:
    images = np.ascontiguousarray(np.asarray(images), dtype=np.float32)
    theta = np.asarray(theta).astype(np.float32)
    out, _ = run_kernel_spmd(images, theta, trace=False)
    return out
